# revision 14
# baseline (speedup 1.0000x reference)
"""Distributed brute-force KNN (retrieval) kernel for one TRN2 chip (8 NeuronCores).

Problem: queries [256,128] f32, candidates [500000,128] f32, identifiers [500000] i32,
k=100. Output: (values [256,100] f32 desc-sorted, ids [256,100] i32).

Strategy:
  - Shard candidates over N across the 8 cores (62500 each, zero-padded to
    123*512 on host; pad claims are filtered out on host).
  - Per core: bf16 matmul (Q stationary, C^T shard streamed) -> PSUM score
    tiles [128q, 512c]. VectorE folds each tile 512->256 (fp32 from PSUM,
    bf16 out) ->128->64 with pairwise max (bf16 at 2x). Each folded slot
    covers a group of 8 candidates. max/max_index extract the top-8
    (value, slot) per 64-slot window per query-half. Claims accumulate in
    SBUF, one DMA out.
  - Host: expand each claimed slot to its 8 candidates, rescore contenders
    exactly in f64, and validate: any window whose 8th claimed value (or a
    duplicated claimed slot) could still hide a top-k element is fully
    rescanned on host. Exactness never depends on device numerics.
"""
import numpy as np
import ml_dtypes

B = 256          # queries
N = 500000       # candidates
D = 128          # dim
NCORES = 8
NSH = N // NCORES          # 62500 real candidates per core
TILE = 512                 # candidates per psum tile
NTILES = 123               # ceil(62500/512)
NSHP = NTILES * TILE       # 62976 padded per core
FOLD = 8                   # candidates per claimed slot (three pairwise folds)
SLOTS = TILE // FOLD       # 64 slots per tile window
CLAIM = NTILES * 8         # claimed entries per (core, query-half) window row

_CACHE = {}


def build(ntiles=NTILES, loops=1, variant="fold3"):
    """Build + compile the per-core Bass program. Returns the compiled Bacc."""
    import concourse.bass as bass
    import concourse.tile as tile
    from concourse import bacc, mybir

    bf16 = mybir.dt.bfloat16
    f32 = mybir.dt.float32
    u16 = mybir.dt.uint16
    Copy = mybir.ActivationFunctionType.Copy
    nsh = ntiles * TILE

    nc = bacc.Bacc("TRN2", debug=False)
    qt = nc.dram_tensor("qt", [D, B], bf16, kind="ExternalInput").ap()
    ct = nc.dram_tensor("ct", [D, nsh], bf16, kind="ExternalInput").ap()
    v8 = nc.dram_tensor("v8", [B, ntiles * 8], bf16, kind="ExternalOutput").ap()
    i8 = nc.dram_tensor("i8", [B, ntiles * 8], u16, kind="ExternalOutput").ap()

    CHUNK = 4  # ct tiles per DMA
    with tile.TileContext(nc) as tc:
        with (
            tc.tile_pool(name="qpool", bufs=1) as qpool,
            tc.tile_pool(name="cpool", bufs=3) as cpool,
            tc.tile_pool(name="psum", bufs=8, space="PSUM") as pp,
            tc.tile_pool(name="fold", bufs=4) as fpool,
            tc.tile_pool(name="acc", bufs=1) as accp,
        ):
            qtile = qpool.tile([D, B], bf16)
            nc.sync.dma_start(qtile[:], qt[:])
            vacc = [
                accp.tile([128, ntiles * 8], bf16, tag=f"vacc{h}", name=f"vacc{h}")
                for h in range(2)
            ]
            iacc = [
                accp.tile([128, ntiles * 8], u16, tag=f"iacc{h}", name=f"iacc{h}")
                for h in range(2)
            ]

            def body(_iv=None):
                for tt in range(0, ntiles, CHUNK):
                    nct = min(CHUNK, ntiles - tt)
                    ctile = cpool.tile([D, CHUNK * TILE], bf16, tag="ct", name="ctile")
                    nc.sync.dma_start(
                        ctile[:, 0 : nct * TILE],
                        ct[:, bass.ds(tt * TILE, nct * TILE)],
                    )
                    for j in range(nct):
                        t = tt + j
                        for h in range(2):
                            ps = pp.tile([128, TILE], f32, name="ps")
                            nc.tensor.matmul(
                                ps[:],
                                lhsT=qtile[:, bass.ds(h * 128, 128)],
                                rhs=ctile[:, bass.ds(j * TILE, TILE)],
                                start=True,
                                stop=True,
                            )
                            vout = vacc[h][:, bass.ds(t * 8, 8)]
                            iout = iacc[h][:, bass.ds(t * 8, 8)]
                            rh = fpool.tile([128, TILE // 2], f32, tag="rh", name="rh")
                            nc.scalar.activation(
                                rh[:], ps[:, bass.ds(256, 256)], Copy
                            )
                            f1 = fpool.tile([128, TILE // 2], bf16, tag="f1", name="f1")
                            nc.vector.tensor_max(f1[:], ps[:, bass.ds(0, 256)], rh[:])
                            f2 = fpool.tile([128, TILE // 4], bf16, tag="f2", name="f2")
                            nc.vector.tensor_max(
                                f2[:], f1[:, bass.ds(0, 128)], f1[:, bass.ds(128, 128)]
                            )
                            f3 = fpool.tile([128, SLOTS], bf16, tag="f3", name="f3")
                            nc.vector.tensor_max(
                                f3[:], f2[:, bass.ds(0, 64)], f2[:, bass.ds(64, 64)]
                            )
                            nc.vector.max(vout, f3[:])
                            nc.vector.max_index(iout, vout, f3[:])

            if loops == 1:
                body()
            else:
                with tc.For_i(0, loops, 1) as iv:
                    body(iv)

            for h in range(2):
                nc.sync.dma_start(v8[bass.ds(h * 128, 128), :], vacc[h][:])
                nc.sync.dma_start(i8[bass.ds(h * 128, 128), :], iacc[h][:])
    nc.compile()
    return nc


def _get_nc():
    if "nc" not in _CACHE:
        _CACHE["nc"] = build()
    return _CACHE["nc"]


def make_in_maps(queries, candidates):
    qt = np.ascontiguousarray(queries.T).astype(ml_dtypes.bfloat16)
    cb = candidates.astype(ml_dtypes.bfloat16)
    in_maps = []
    for c in range(NCORES):
        ct = np.zeros((D, NSHP), dtype=ml_dtypes.bfloat16)
        ct[:, :NSH] = cb[c * NSH : (c + 1) * NSH].T
        in_maps.append({"qt": qt, "ct": ct})
    return in_maps


def _device_claims(queries, candidates):
    """Run the 8-core SPMD kernel; return claimed (vals, slot base gidx) arrays."""
    from concourse.bass_utils import run_bass_kernel_spmd

    nc = _get_nc()
    in_maps = make_in_maps(queries, candidates)
    res = None
    for attempt in range(3):
        try:
            res = run_bass_kernel_spmd(nc, in_maps, core_ids=list(range(NCORES))).results
            break
        except Exception:
            if attempt == 2:
                raise
            import time as _time

            _time.sleep(2.0)
    assert res is not None
    v8 = np.stack([r["v8"] for r in res]).astype(np.float32)   # [8, B, CLAIM]
    i8 = np.stack([r["i8"] for r in res]).astype(np.int64)     # [8, B, CLAIM] slot in [0,SLOTS)
    # padded-local base index of the claimed slot (member m adds m*SLOTS):
    offs = (np.arange(CLAIM) // 8) * TILE
    lbase = i8 + offs[None, None, :]                           # local in [0, NSHP)
    return v8, i8, lbase


def _expand_local(lb):
    """Expand local slot bases [...] -> FOLD local member indices [..., FOLD]."""
    return lb[..., None] + (np.arange(FOLD) * SLOTS)[None, :]


def kernel(queries, candidates, identifiers, k):
    queries = np.asarray(queries, dtype=np.float32)
    candidates = np.asarray(candidates, dtype=np.float32)
    identifiers = np.asarray(identifiers)
    kk = int(k)

    v8, i8, lbase = _device_claims(queries, candidates)
    core_off = (np.arange(NCORES) * NSH)[:, None, None]

    # flatten claims to [B, NCORES*CLAIM]
    vals = v8.transpose(1, 0, 2).reshape(B, NCORES * CLAIM)
    lflat = lbase.transpose(1, 0, 2).reshape(B, NCORES * CLAIM)
    cflat = np.broadcast_to(
        np.arange(NCORES)[None, :, None], (B, NCORES, CLAIM)
    ).reshape(B, NCORES * CLAIM)

    q64 = queries.astype(np.float64)
    sigma = np.linalg.norm(queries, axis=1)

    def rescore_members(lb, cores, q):
        """lb: local slot bases [M], cores [M] -> exact scores + global ids."""
        mem = _expand_local(lb)                       # [M, FOLD] local padded idx
        valid = mem < NSH
        gl = mem + cores[:, None] * NSH               # global real idx (where valid)
        gl_f = np.where(valid, gl, 0)
        sv = candidates[gl_f].astype(np.float64) @ q64[q]
        sv = np.where(valid, sv, -np.inf)
        return sv.ravel(), np.where(valid, gl, -1).ravel()

    # --- preselect top-C claims per query, rescore their groups exactly ---
    C = max(2 * kk, kk + 64)
    part = np.argpartition(-vals, C, axis=1)[:, :C]
    vsel = np.take_along_axis(vals, part, 1)
    lsel = np.take_along_axis(lflat, part, 1)
    csel = np.take_along_axis(cflat, part, 1)
    mem = _expand_local(lsel)                          # [B, C, FOLD]
    valid = mem < NSH
    gsel = np.where(valid, mem + csel[..., None] * NSH, 0)
    se = np.einsum("qcd,qd->qc", candidates[gsel.reshape(B, -1)].astype(np.float64), q64)
    se = np.where(valid.reshape(B, -1), se, -np.inf)
    se_g = se.reshape(B, C, FOLD)
    # device claim error bound per query (claim ~ max over group's exact scores)
    gmax = se_g.max(2)
    finite = np.isfinite(gmax)
    delta = np.where(finite, np.abs(vsel - gmax), 0.0).max(1)
    margin = 4.0 * delta + 1e-3 * sigma

    vk = -np.partition(-se, kk - 1, axis=1)[:, kk - 1]
    thr = vk - margin

    pool_v = [se[q] for q in range(B)]
    pool_g = [np.where(valid, mem + csel[..., None] * NSH, -1)[q].ravel() for q in range(B)]

    # 1) any claimed entry above thr that wasn't rescored
    selmask = np.zeros(vals.shape, dtype=bool)
    np.put_along_axis(selmask, part, True, 1)
    need = (vals >= thr[:, None]) & ~selmask
    for q in np.nonzero(need.any(1))[0]:
        sv, gl = rescore_members(lflat[q, need[q]], cflat[q, need[q]], q)
        pool_v[q] = np.concatenate([pool_v[q], sv])
        pool_g[q] = np.concatenate([pool_g[q], gl])

    # 2) suspect windows: (a) 8th claimed value could hide an unclaimed slot,
    #    (b) duplicated claimed slot (f32/bf16 value tie collapsing groups)
    tmin = v8[:, :, 7::8]                              # [8, B, NTILES]
    sus = tmin >= (thr - margin)[None, :, None]
    iw = np.sort(i8.reshape(NCORES, B, NTILES, 8), axis=3)
    hasdup = (np.diff(iw, axis=3) == 0).any(3)         # [8, B, NTILES]
    vmax_w = v8[:, :, 0::8]
    sus |= hasdup & (vmax_w >= (thr - margin)[None, :, None])
    for q, c, t in zip(*np.nonzero(sus.transpose(1, 0, 2))):
        base = t * TILE
        hi = min(base + TILE, NSH)
        if hi <= base:
            continue
        gb = c * NSH + base
        sv = candidates[gb : c * NSH + hi].astype(np.float64) @ q64[q]
        g = np.arange(gb, c * NSH + hi, dtype=np.int64)
        pool_v[q] = np.concatenate([pool_v[q], sv])
        pool_g[q] = np.concatenate([pool_g[q], g])

    # --- final exact top-k per query (dedupe, desc value, index tiebreak) --
    out_v = np.empty((B, kk), np.float32)
    out_g = np.empty((B, kk), np.int64)
    for q in range(B):
        keep = pool_g[q] >= 0
        g, first = np.unique(pool_g[q][keep], return_index=True)
        v32 = pool_v[q][keep][first].astype(np.float32)
        assert v32.size >= kk
        order = np.lexsort((g, -v32))[:kk]
        out_v[q] = v32[order]
        out_g[q] = g[order]

    top_ids = identifiers[out_g]
    return out_v, top_ids


# revision 15
# speedup vs baseline: 1.0413x; 1.0413x over previous
"""Distributed brute-force KNN (retrieval) kernel for one TRN2 chip (8 NeuronCores).

Problem: queries [256,128] f32, candidates [500000,128] f32, identifiers [500000] i32,
k=100. Output: (values [256,100] f32 desc-sorted, ids [256,100] i32).

Strategy:
  - Shard candidates over N across the 8 cores (62500 each, zero-padded to
    123*512 on host; pad claims are filtered out on host).
  - Per core: bf16 matmul (Q stationary, C^T shard streamed) -> PSUM score
    tiles [128q, 512c]. VectorE folds each tile 512->256 (fp32 from PSUM,
    bf16 out) ->128->64 with pairwise max (bf16 at 2x). Each folded slot
    covers a group of 8 candidates. max/max_index extract the top-8
    (value, slot) per 64-slot window per query-half. Claims accumulate in
    SBUF, one DMA out.
  - Host: expand each claimed slot to its 8 candidates, rescore contenders
    exactly in f64, and validate: any window whose 8th claimed value (or a
    duplicated claimed slot) could still hide a top-k element is fully
    rescanned on host. Exactness never depends on device numerics.
"""
import numpy as np
import ml_dtypes

B = 256          # queries
N = 500000       # candidates
D = 128          # dim
NCORES = 8
NSH = N // NCORES          # 62500 real candidates per core
TILE = 500                 # candidates per psum tile
NTILES = 125               # 62500/500
NSHP = NTILES * TILE       # == NSH (no padding needed)
FOLD = 4                   # candidates per claimed slot (two pairwise folds)
SLOTS = TILE // FOLD       # 125 slots per tile window
CLAIM = NTILES * 8         # claimed entries per (core, query-half) window row

_CACHE = {}


def build(ntiles=NTILES, loops=1, variant="fold3"):
    """Build + compile the per-core Bass program. Returns the compiled Bacc."""
    import concourse.bass as bass
    import concourse.tile as tile
    from concourse import bacc, mybir

    bf16 = mybir.dt.bfloat16
    f32 = mybir.dt.float32
    u16 = mybir.dt.uint16
    Copy = mybir.ActivationFunctionType.Copy
    nsh = ntiles * TILE

    nc = bacc.Bacc("TRN2", debug=False)
    qt = nc.dram_tensor("qt", [D, B], bf16, kind="ExternalInput").ap()
    ct = nc.dram_tensor("ct", [D, nsh], bf16, kind="ExternalInput").ap()
    v8 = nc.dram_tensor("v8", [B, ntiles * 8], f32, kind="ExternalOutput").ap()
    i8 = nc.dram_tensor("i8", [B, ntiles * 8], u16, kind="ExternalOutput").ap()

    CHUNK = 4  # ct tiles per DMA
    with tile.TileContext(nc) as tc:
        with (
            tc.tile_pool(name="qpool", bufs=1) as qpool,
            tc.tile_pool(name="cpool", bufs=3) as cpool,
            tc.tile_pool(name="psum", bufs=8, space="PSUM") as pp,
            tc.tile_pool(name="fold", bufs=4) as fpool,
            tc.tile_pool(name="acc", bufs=1) as accp,
        ):
            qtile = qpool.tile([D, B], bf16)
            nc.sync.dma_start(qtile[:], qt[:])
            vacc = [
                accp.tile([128, ntiles * 8], f32, tag=f"vacc{h}", name=f"vacc{h}")
                for h in range(2)
            ]
            iacc = [
                accp.tile([128, ntiles * 8], u16, tag=f"iacc{h}", name=f"iacc{h}")
                for h in range(2)
            ]

            def body(_iv=None):
                for tt in range(0, ntiles, CHUNK):
                    nct = min(CHUNK, ntiles - tt)
                    ctile = cpool.tile([D, CHUNK * TILE], bf16, tag="ct", name="ctile")
                    nc.sync.dma_start(
                        ctile[:, 0 : nct * TILE],
                        ct[:, bass.ds(tt * TILE, nct * TILE)],
                    )
                    for j in range(nct):
                        t = tt + j
                        for h in range(2):
                            ps = pp.tile([128, TILE], f32, name="ps")
                            nc.tensor.matmul(
                                ps[:],
                                lhsT=qtile[:, bass.ds(h * 128, 128)],
                                rhs=ctile[:, bass.ds(j * TILE, TILE)],
                                start=True,
                                stop=True,
                            )
                            vout = vacc[h][:, bass.ds(t * 8, 8)]
                            iout = iacc[h][:, bass.ds(t * 8, 8)]
                            f0 = fpool.tile([128, TILE], f32, tag="f0", name="f0")
                            nc.scalar.activation(f0[:], ps[:], Copy)
                            f1 = fpool.tile([128, TILE // 2], f32, tag="f1", name="f1")
                            nc.vector.tensor_max(
                                f1[:], f0[:, bass.ds(0, 250)], f0[:, bass.ds(250, 250)]
                            )
                            f2 = fpool.tile([128, SLOTS], f32, tag="f2", name="f2")
                            nc.vector.tensor_max(
                                f2[:], f1[:, bass.ds(0, 125)], f1[:, bass.ds(125, 125)]
                            )
                            nc.vector.max(vout, f2[:])
                            nc.vector.max_index(iout, vout, f2[:])

            if loops == 1:
                body()
            else:
                with tc.For_i(0, loops, 1) as iv:
                    body(iv)

            for h in range(2):
                nc.sync.dma_start(v8[bass.ds(h * 128, 128), :], vacc[h][:])
                nc.sync.dma_start(i8[bass.ds(h * 128, 128), :], iacc[h][:])
    nc.compile()
    return nc


def _get_nc():
    if "nc" not in _CACHE:
        _CACHE["nc"] = build()
    return _CACHE["nc"]


def make_in_maps(queries, candidates):
    qt = np.ascontiguousarray(queries.T).astype(ml_dtypes.bfloat16)
    cb = candidates.astype(ml_dtypes.bfloat16)
    in_maps = []
    for c in range(NCORES):
        ct = np.zeros((D, NSHP), dtype=ml_dtypes.bfloat16)
        ct[:, :NSH] = cb[c * NSH : (c + 1) * NSH].T
        in_maps.append({"qt": qt, "ct": ct})
    return in_maps


def _device_claims(queries, candidates):
    """Run the 8-core SPMD kernel; return claimed (vals, slot base gidx) arrays."""
    from concourse.bass_utils import run_bass_kernel_spmd

    nc = _get_nc()
    in_maps = make_in_maps(queries, candidates)
    res = None
    for attempt in range(3):
        try:
            res = run_bass_kernel_spmd(nc, in_maps, core_ids=list(range(NCORES))).results
            break
        except Exception:
            if attempt == 2:
                raise
            import time as _time

            _time.sleep(2.0)
    assert res is not None
    v8 = np.stack([r["v8"] for r in res]).astype(np.float32)   # [8, B, CLAIM]
    i8 = np.stack([r["i8"] for r in res]).astype(np.int64)     # [8, B, CLAIM] slot in [0,SLOTS)
    # padded-local base index of the claimed slot (member m adds m*SLOTS):
    offs = (np.arange(CLAIM) // 8) * TILE
    lbase = i8 + offs[None, None, :]                           # local in [0, NSHP)
    return v8, i8, lbase


def _expand_local(lb):
    """Expand local slot bases [...] -> FOLD local member indices [..., FOLD]."""
    return lb[..., None] + (np.arange(FOLD) * SLOTS)[None, :]


def kernel(queries, candidates, identifiers, k):
    queries = np.asarray(queries, dtype=np.float32)
    candidates = np.asarray(candidates, dtype=np.float32)
    identifiers = np.asarray(identifiers)
    kk = int(k)

    v8, i8, lbase = _device_claims(queries, candidates)
    core_off = (np.arange(NCORES) * NSH)[:, None, None]

    # flatten claims to [B, NCORES*CLAIM]
    vals = v8.transpose(1, 0, 2).reshape(B, NCORES * CLAIM)
    lflat = lbase.transpose(1, 0, 2).reshape(B, NCORES * CLAIM)
    cflat = np.broadcast_to(
        np.arange(NCORES)[None, :, None], (B, NCORES, CLAIM)
    ).reshape(B, NCORES * CLAIM)

    q64 = queries.astype(np.float64)
    sigma = np.linalg.norm(queries, axis=1)

    def rescore_members(lb, cores, q):
        """lb: local slot bases [M], cores [M] -> exact scores + global ids."""
        mem = _expand_local(lb)                       # [M, FOLD] local padded idx
        valid = mem < NSH
        gl = mem + cores[:, None] * NSH               # global real idx (where valid)
        gl_f = np.where(valid, gl, 0)
        sv = candidates[gl_f].astype(np.float64) @ q64[q]
        sv = np.where(valid, sv, -np.inf)
        return sv.ravel(), np.where(valid, gl, -1).ravel()

    # --- preselect top-C claims per query, rescore their groups exactly ---
    C = max(2 * kk, kk + 64)
    part = np.argpartition(-vals, C, axis=1)[:, :C]
    vsel = np.take_along_axis(vals, part, 1)
    lsel = np.take_along_axis(lflat, part, 1)
    csel = np.take_along_axis(cflat, part, 1)
    mem = _expand_local(lsel)                          # [B, C, FOLD]
    valid = mem < NSH
    gsel = np.where(valid, mem + csel[..., None] * NSH, 0)
    se = np.einsum("qcd,qd->qc", candidates[gsel.reshape(B, -1)].astype(np.float64), q64)
    se = np.where(valid.reshape(B, -1), se, -np.inf)
    se_g = se.reshape(B, C, FOLD)
    # device claim error bound per query (claim ~ max over group's exact scores)
    gmax = se_g.max(2)
    finite = np.isfinite(gmax)
    delta = np.where(finite, np.abs(vsel - gmax), 0.0).max(1)
    margin = 4.0 * delta + 1e-3 * sigma

    vk = -np.partition(-se, kk - 1, axis=1)[:, kk - 1]
    thr = vk - margin

    pool_v = [se[q] for q in range(B)]
    pool_g = [np.where(valid, mem + csel[..., None] * NSH, -1)[q].ravel() for q in range(B)]

    # 1) any claimed entry above thr that wasn't rescored
    selmask = np.zeros(vals.shape, dtype=bool)
    np.put_along_axis(selmask, part, True, 1)
    need = (vals >= thr[:, None]) & ~selmask
    for q in np.nonzero(need.any(1))[0]:
        sv, gl = rescore_members(lflat[q, need[q]], cflat[q, need[q]], q)
        pool_v[q] = np.concatenate([pool_v[q], sv])
        pool_g[q] = np.concatenate([pool_g[q], gl])

    # 2) suspect windows: (a) 8th claimed value could hide an unclaimed slot,
    #    (b) duplicated claimed slot (f32/bf16 value tie collapsing groups)
    tmin = v8[:, :, 7::8]                              # [8, B, NTILES]
    sus = tmin >= (thr - margin)[None, :, None]
    iw = np.sort(i8.reshape(NCORES, B, NTILES, 8), axis=3)
    hasdup = (np.diff(iw, axis=3) == 0).any(3)         # [8, B, NTILES]
    vmax_w = v8[:, :, 0::8]
    sus |= hasdup & (vmax_w >= (thr - margin)[None, :, None])
    for q, c, t in zip(*np.nonzero(sus.transpose(1, 0, 2))):
        base = t * TILE
        hi = min(base + TILE, NSH)
        if hi <= base:
            continue
        gb = c * NSH + base
        sv = candidates[gb : c * NSH + hi].astype(np.float64) @ q64[q]
        g = np.arange(gb, c * NSH + hi, dtype=np.int64)
        pool_v[q] = np.concatenate([pool_v[q], sv])
        pool_g[q] = np.concatenate([pool_g[q], g])

    # --- final exact top-k per query (dedupe, desc value, index tiebreak) --
    out_v = np.empty((B, kk), np.float32)
    out_g = np.empty((B, kk), np.int64)
    for q in range(B):
        keep = pool_g[q] >= 0
        g, first = np.unique(pool_g[q][keep], return_index=True)
        v32 = pool_v[q][keep][first].astype(np.float32)
        assert v32.size >= kk
        order = np.lexsort((g, -v32))[:kk]
        out_v[q] = v32[order]
        out_g[q] = g[order]

    top_ids = identifiers[out_g]
    return out_v, top_ids
